# revision 73
# baseline (speedup 1.0000x reference)
"""Trainium2 Bass kernel for the DNA GNN (nn_DNA_65360812310552).

Strategy (8 NeuronCores, SPMD):
  - Nodes padded to NP=10240, sharded by col-range: core c owns nodes
    [c*1280, (c+1)*1280) and ALL edges whose target (col) lies in that
    range.  Aggregation is therefore core-local: no reduce collectives.
  - Per layer, per 128-node window, per-edge attention messages are
    computed in edge-partition layout and segment-summed into the window
    via a selection-matrix matmul on the TensorEngine (PSUM accumulate),
    which is exact and collision-free.
  - Node tables (layer history T_l and transformed queries QT) are
    [NP, C] tensors in DRAM, AllGathered each layer; per-edge rows are
    fetched with gpsimd dma_gather.
  - Algebra (validated in numpy): the key bias term bk cancels in
    softmax; Wk is folded into the query (qt = glinT(q, Wk)/sqrt(CH));
    Wv+bv are deferred past the attention + segment-sum; the gcn norm
    dis[row]*dis[col] is split: dis[row] scales messages, dis[col] is
    applied after aggregation; S = segsum(dis[row]) provides the bias
    path (agg = dis * (glin(U, Wv) + S*bv)).

Self-contained: hardcodes all shapes; builds the Bass program per input
(edge partition sizes are baked in), runs via run_bass_kernel_spmd on
cores 0-7, reassembles the full [10000, 16] output.
"""

import ml_dtypes
import numpy as np

import concourse.bacc as bacc
import concourse.bass as bass
import concourse.mybir as mybir
import concourse.tile as tile
from concourse.bass_utils import run_bass_kernel_spmd
from concourse.masks import make_identity

# problem constants
N = 10000
E = 160000
C = 128
H = 8
CH = 16
G = 16
CG = 8
L = 5
NF = 14
NFP = 16          # NF padded
DOUT = 16
NCORES = 8

NP = 10240        # padded node count = 8 * 1280
NSL = NP // NCORES  # 1280 nodes per core
NW = NSL // 128     # 10 windows of 128 nodes per core
NB = 8              # max tiles (of 128 edges) per chunk

F32 = mybir.dt.float32
BF16 = mybir.dt.bfloat16
I16 = mybir.dt.int16

# edge-pipeline dtype (tables, gathers, products).  f32 is exact;
# bf16 halves DMA+DVE cost.
EDT = mybir.dt.bfloat16
EDT_NP = np.float32 if EDT == F32 else np.dtype("bfloat16") if hasattr(np, "bfloat16") else None


def _wrap_idx(a: np.ndarray) -> np.ndarray:
    """[T] int -> [128, T//16] int16 in dma_gather's wrapped layout:
    idx j lives at partition j%16, column j//16, replicated 8x."""
    T = a.shape[0]
    assert T % 16 == 0
    w = a.reshape(T // 16, 16).T.astype(np.int16)  # [16, T//16]
    return np.tile(w, (8, 1))                       # [128, T//16]


def _chunks(nt: int) -> list[int]:
    k = -(-nt // NB)
    base = nt // k
    out = [base] * k
    for i in range(nt - base * k):
        out[i] += 1
    return out


def build_program(tiles_w: list[int], skip=frozenset(), reps=1):
    """Build the SPMD Bass program.  tiles_w[w] = number of 128-edge
    tiles in window w (identical across cores, host-padded)."""
    TOT = sum(tiles_w) * 128          # padded edges per core
    NTIL = sum(tiles_w)
    NTX = max(tiles_w)                # tiles in the largest window

    nc = bacc.Bacc("TRN2", target_bir_lowering=False, debug=False,
                   num_devices=NCORES)

    # ---- I/O ----
    xsl = nc.dram_tensor("xsl", [NSL, NFP], F32, kind="ExternalInput")
    rowi = nc.dram_tensor("rowi", [128, TOT // 16], I16, kind="ExternalInput")
    coli = nc.dram_tensor("coli", [128, TOT // 16], I16, kind="ExternalInput")
    colw_d = nc.dram_tensor("colw", [TOT], EDT, kind="ExternalInput")
    disrow_d = nc.dram_tensor("disrow", [TOT], F32, kind="ExternalInput")
    st_d = nc.dram_tensor("st", [NSL], F32, kind="ExternalInput")
    disn_d = nc.dram_tensor("disn", [NSL], F32, kind="ExternalInput")
    w1_d = nc.dram_tensor("w1", [NFP, C], F32, kind="ExternalInput")
    b1_d = nc.dram_tensor("b1", [C], F32, kind="ExternalInput")
    wq_d = nc.dram_tensor("wq", [L, C, C], F32, kind="ExternalInput")
    wkt_d = nc.dram_tensor("wkt", [L, C, C], F32, kind="ExternalInput")
    wv_d = nc.dram_tensor("wv", [L, C, C], F32, kind="ExternalInput")
    bq_d = nc.dram_tensor("bq", [L, C], F32, kind="ExternalInput")
    bv_d = nc.dram_tensor("bv", [L, C], F32, kind="ExternalInput")
    l2w_d = nc.dram_tensor("l2w", [C, DOUT], F32, kind="ExternalInput")
    l2b_d = nc.dram_tensor("l2b", [DOUT], F32, kind="ExternalInput")
    y_d = nc.dram_tensor("y", [NSL, DOUT], F32, kind="ExternalOutput")

    # ---- internal DRAM ----
    xsl_d = nc.dram_tensor("xsl_int", [NSL, C], EDT)     # AG input (x_l slice)
    tq_f = nc.dram_tensor("tq_f", [NP, L * C], EDT)   # packed x0..x4 rows
    xf_b = nc.dram_tensor("xf_b", [NP, C], EDT, addr_space="Shared")

    groups = [list(range(NCORES))]

    with tile.TileContext(nc) as tc:
        with (
            tc.tile_pool(name="const", bufs=1) as cpool,
            tc.tile_pool(name="work", bufs=2) as pool,
            tc.tile_pool(name="psum", bufs=2, space="PSUM") as psp,
            tc.tile_pool(name="psw", bufs=2, space="PSUM") as pswp,
            tc.tile_pool(name="psq", bufs=1, space="PSUM") as psqp,
        ):
            # ---------- constants ----------
            ident = cpool.tile([128, 128], F32)
            make_identity(nc, ident[:])
            ident_b = cpool.tile([128, 128], EDT)
            nc.vector.tensor_copy(ident_b[:], ident[:])
            iota_i = cpool.tile([128, 128], mybir.dt.int32)
            nc.gpsimd.iota(iota_i[:], pattern=[[1, 128]], base=0,
                           channel_multiplier=0)
            iotaf = cpool.tile([128, 128], EDT)
            nc.vector.tensor_copy(iotaf[:], iota_i[:])

            # selection tiles S[e, t, n] = (colw[t*128+e] == n)
            s_all = cpool.tile([128, NTIL, 128], EDT)
            # transposed selection tiles St[n, t, e] (for PE q-expansion)
            st_all = cpool.tile([128, NTIL, 128], EDT)
            # qt table for all local nodes, node-major [128, NW*C]
            qt_all = cpool.tile([128, NW * C], EDT)
            colwv_all = cpool.tile([128, NTIL], EDT)
            nc.sync.dma_start(
                out=colwv_all[:],
                in_=colw_d[:].rearrange("(b p) -> p b", p=128))

            w1_sb = cpool.tile([NFP, C], F32)
            nc.sync.dma_start(out=w1_sb[:], in_=w1_d[:])
            b1_sb = cpool.tile([C, 1], F32)
            nc.sync.dma_start(out=b1_sb[:], in_=b1_d[:, None])
            l2w_sb = cpool.tile([C, DOUT], F32)
            nc.sync.dma_start(out=l2w_sb[:], in_=l2w_d[:])
            l2b_sb = cpool.tile([1, DOUT], F32)
            nc.sync.dma_start(out=l2b_sb[:], in_=l2b_d[:][None, :])
            ones_row = cpool.tile([1, 128], F32)
            nc.gpsimd.memset(ones_row[:], 1.0)
            ones_col = cpool.tile([128, 1], EDT)
            nc.gpsimd.memset(ones_col[:], 1.0)

            rowi_sb = cpool.tile([128, TOT // 16], I16)
            nc.sync.dma_start(out=rowi_sb[:], in_=rowi[:])

            disrow = cpool.tile([128, NTIL], F32)   # dis[row[e]] per edge
            nc.sync.dma_start(
                out=disrow[:],
                in_=disrow_d[:].rearrange("(b p) -> p b", p=128))
            dis_np = cpool.tile([128, NW], F32)
            nc.sync.dma_start(
                out=dis_np[:],
                in_=disn_d[:].rearrange("(w p) -> p w", p=128))
            xc_all = cpool.tile([128, NSL], F32)    # current x_l, c-part

            # reps>1 re-runs the whole computation for timing-by-scaling
            for _rep in range(reps):
              # ------- S_all: selection tiles, once per rep -------
              nc.vector.tensor_tensor(
                  out=s_all[:],
                  in0=colwv_all[:].unsqueeze(2).to_broadcast([128, NTIL, 128]),
                  in1=iotaf[:].unsqueeze(1).to_broadcast([128, NTIL, 128]),
                  op=mybir.AluOpType.is_equal,
              )
              # St_all = per-tile transposes of S_all (PE + scalar copies)
              for t in range(NTIL):
                  pst = psp.tile([128, 128], EDT, tag="pnt")
                  nc.tensor.transpose(pst[:], s_all[:, t, :], ident_b[:])
                  nc.scalar.activation(
                      out=st_all[:, t, :], in_=pst[:],
                      func=mybir.ActivationFunctionType.Copy)



              # ---------- x0 = relu(x @ W1 + b1) ----------
              for ch in range(NW):
                  xin = pool.tile([128, NFP], F32, tag="xin")
                  nc.sync.dma_start(out=xin[:],
                                    in_=xsl[ch * 128:(ch + 1) * 128, :])
                  pxt = psp.tile([NFP, 128], F32, tag="pnt")
                  nc.tensor.transpose(pxt[:], xin[:], ident[:])
                  xt = pool.tile([NFP, 128], F32, tag="xt")
                  nc.vector.tensor_copy(xt[:], pxt[:])
                  pm = psp.tile([128, 128], F32, tag="pm")
                  nc.tensor.matmul(pm[:], lhsT=w1_sb[:], rhs=xt[:],
                                   start=True, stop=True)
                  nc.scalar.activation(
                      out=xc_all[:, ch * 128:(ch + 1) * 128], in_=pm[:],
                      func=mybir.ActivationFunctionType.Relu,
                      bias=b1_sb[:, 0:1])
                  pnt = psp.tile([128, 128], F32, tag="pnt")
                  nc.tensor.transpose(pnt[:], xc_all[:, ch * 128:(ch + 1) * 128],
                                      ident[:])
                  xn = pool.tile([128, C], EDT, tag="xn")
                  nc.vector.tensor_copy(xn[:], pnt[:])
                  nc.sync.dma_start(out=xsl_d[ch * 128:(ch + 1) * 128, :],
                                    in_=xn[:])
              if "cc" not in skip:
                  nc.gpsimd.collective_compute(
                      "AllGather", mybir.AluOpType.bypass, replica_groups=groups,
                      ins=[xsl_d[:]], outs=[xf_b[:]])
              nc.sync.dma_start(out=tq_f[:, 0:C], in_=xf_b[:])

              # ---------- layers ----------
              for l in range(L):
                  Lc = l + 1
                  wq_sb = pool.tile([128, 128], F32, tag="wq_sb")
                  nc.sync.dma_start(out=wq_sb[:], in_=wq_d[l])
                  wkt_sb = pool.tile([128, 128], F32, tag="wkt_sb")
                  nc.sync.dma_start(out=wkt_sb[:], in_=wkt_d[l])
                  wv_sb = pool.tile([128, 128], F32, tag="wv_sb")
                  nc.sync.dma_start(out=wv_sb[:], in_=wv_d[l])
                  bq_sb = pool.tile([C, 1], F32, tag="bq_sb")
                  nc.sync.dma_start(out=bq_sb[:],
                                    in_=bq_d[l][:, None])
                  bv_row = pool.tile([1, C], F32, tag="bv_row")
                  nc.sync.dma_start(out=bv_row[:],
                                    in_=bv_d[l][None, :])

                  # qt = glinT(glin(x_l, Wq)+bq, Wk) / 4, from xc_all (c-part)
                  # (layer 0 has softmax==1: no queries needed)
                  for ch in range(NW if l > 0 else 0):
                      pq = psp.tile([128, 128], F32, tag="pm")
                      nc.tensor.matmul(pq[:], lhsT=wq_sb[:],
                                       rhs=xc_all[:, ch * 128:(ch + 1) * 128],
                                       start=True, stop=True)
                      qs = pool.tile([128, 128], F32, tag="qs", bufs=1)
                      nc.scalar.activation(
                          out=qs[:], in_=pq[:],
                          func=mybir.ActivationFunctionType.Identity,
                          bias=bq_sb[:, 0:1])
                      pq2 = psp.tile([128, 128], F32, tag="pm")
                      nc.tensor.matmul(pq2[:], lhsT=wkt_sb[:], rhs=qs[:],
                                       start=True, stop=True)
                      qtc = pool.tile([128, 128], F32, tag="qtc", bufs=1)
                      nc.scalar.activation(
                          out=qtc[:], in_=pq2[:],
                          func=mybir.ActivationFunctionType.Copy, scale=0.25)
                      pq3 = psp.tile([128, 128], F32, tag="pnt")
                      nc.tensor.transpose(pq3[:], qtc[:], ident[:])
                      nc.scalar.activation(
                          out=qt_all[:, ch * C:(ch + 1) * C], in_=pq3[:],
                          func=mybir.ActivationFunctionType.Copy)

                  # ---- edge phase: whole-window units, layer-batched DVE ----
                  t0 = 0
                  for w in range(NW):
                      NT = tiles_w[w]
                      upsw = pswp.tile([128, C], F32, tag="acc")
                      if "pe" in skip:
                          nc.vector.memset(upsw[:], 0.0)
                      # qg[e,:] = qt[col_e,:] via St matmuls (layer 0: no q)
                      if l > 0:
                          qg = pool.tile([128, NTX, C], EDT, tag="qg")
                          for p0 in range(0, NT, NB):
                              pn = min(NB, NT - p0)
                              qg_ps = psqp.tile([128, NB, C], F32, tag="qgp")
                              for t in range(pn):
                                  nc.tensor.matmul(
                                      qg_ps[:, t, :],
                                      lhsT=st_all[:, t0 + p0 + t, :],
                                      rhs=qt_all[:, w * C:(w + 1) * C],
                                      start=True, stop=True)
                              nc.scalar.activation(
                                  out=qg[:, p0:p0 + pn, :],
                                  in_=qg_ps[:, :pn, :],
                                  func=mybir.ActivationFunctionType.Copy)
                      # x rows for j=0..l in one gather per edge (tq_f holds
                      # x_l after the layer-entry copy from xf_b)
                      xga = pool.tile([128, NTX, Lc * C], EDT, tag="xh")
                      if "gather" in skip:
                          nc.vector.memset(xga[:, :NT, :], 0.25)
                      if "gather" not in skip:
                          for p0 in range(0, NT, NB):
                              pn = min(NB, NT - p0)
                              idx = rowi_sb[:, (t0 + p0) * 8:(t0 + p0 + pn) * 8]
                              nc.gpsimd.dma_gather(
                                  xga[:, p0:p0 + pn, :], tq_f[:, :Lc * C],
                                  idx, pn * 128, pn * 128, Lc * C,
                                  elem_step=L * C)

                      def xg(j):
                          return xga[:, :NT, j * C:(j + 1) * C]

                      msf = pool.tile([128, NTX, C], EDT, tag="msf", bufs=1)
                      if "dve" in skip:
                          nc.vector.memset(msf[:, :NT, :], 0.25)
                      if "dve" not in skip and l == 0:
                          # softmax over 1 item == 1: msf = disrow * x0
                          wfd = pool.tile([128, NTX, H], EDT, tag="wf", bufs=1)
                          nc.vector.tensor_copy(
                              wfd[:, :NT, :],
                              disrow[:, t0:t0 + NT].unsqueeze(2)
                                  .to_broadcast([128, NT, H]))
                          nc.vector.tensor_tensor(
                              out=msf[:, :NT, :].rearrange(
                                  "p t (c h) -> p t c h", h=H),
                              in0=xg(0).rearrange(
                                  "p t (c h) -> p t c h", h=H),
                              in1=wfd[:, :NT, :].unsqueeze(2)
                                  .to_broadcast([128, NT, CH, H]),
                              op=mybir.AluOpType.mult)
                      if "dve" not in skip and l > 0:
                          # scores: PMt[:,t,j,:] = x_j*q, then c2-halving tree
                          PMt = pool.tile([128, NTX, Lc, C], EDT, tag="PM",
                                          bufs=1)
                          for j in range(Lc):
                              nc.vector.tensor_tensor(
                                  out=PMt[:, :NT, j, :], in0=xg(j),
                                  in1=qg[:, :NT, :], op=mybir.AluOpType.mult)
                          V = PMt[:, :NT, :, :].rearrange(
                              "p t l (c h) -> p (t l) c h", h=H)
                          for half in (8, 4, 2):
                              nc.vector.tensor_tensor(
                                  out=V[:, :, 0:half, :],
                                  in0=V[:, :, 0:half, :],
                                  in1=V[:, :, half:2 * half, :],
                                  op=mybir.AluOpType.add)
                          sc = pool.tile([128, NTX, Lc, H], F32, tag="sc",
                                         bufs=1)
                          nc.vector.tensor_tensor(
                              out=sc[:, :NT, :, :].rearrange(
                                  "p t l h -> p (t l) h"),
                              in0=V[:, :, 0, :], in1=V[:, :, 1, :],
                              op=mybir.AluOpType.add)
                          ex = pool.tile([128, NTX, Lc, H], EDT, tag="ex", bufs=1)
                          nc.scalar.activation(
                              out=ex[:, :NT, :, :].rearrange(
                                  "p t l h -> p (t l) h"),
                              in_=sc[:, :NT, :, :].rearrange(
                                  "p t l h -> p (t l) h"),
                              func=mybir.ActivationFunctionType.Exp)
                          den = pool.tile([128, NTX, H], F32, tag="den",
                                          bufs=1)
                          nc.vector.tensor_tensor(
                              out=den[:, :NT, :], in0=ex[:, :NT, 0, :],
                              in1=ex[:, :NT, 1, :], op=mybir.AluOpType.add)
                          for j in range(2, Lc):
                              nc.vector.tensor_tensor(
                                  out=den[:, :NT, :], in0=den[:, :NT, :],
                                  in1=ex[:, :NT, j, :],
                                  op=mybir.AluOpType.add)
                          rec = pool.tile([128, NTX, H], F32, tag="rec",
                                          bufs=1)
                          nc.vector.reciprocal(rec[:, :NT, :], den[:, :NT, :])
                          wf = pool.tile([128, NTX, H], EDT, tag="wf", bufs=1)
                          nc.vector.tensor_tensor(
                              out=wf[:, :NT, :], in0=rec[:, :NT, :],
                              in1=disrow[:, t0:t0 + NT].unsqueeze(2)
                                  .to_broadcast([128, NT, H]),
                              op=mybir.AluOpType.mult)
                          # fold wf into the (tiny) weights: ex *= wf
                          nc.vector.tensor_tensor(
                              out=ex[:, :NT, :, :], in0=ex[:, :NT, :, :],
                              in1=wf[:, :NT, :].unsqueeze(2)
                                  .to_broadcast([128, NT, Lc, H]),
                              op=mybir.AluOpType.mult)
                          # messages: msf = sum_j (ex_j*wf) * x_j
                          for j in range(Lc):
                              nc.vector.tensor_tensor(
                                  out=PMt[:, :NT, j, :].rearrange(
                                      "p t (c h) -> p t c h", h=H),
                                  in0=xg(j).rearrange(
                                      "p t (c h) -> p t c h", h=H),
                                  in1=ex[:, :NT, j, :].unsqueeze(2)
                                      .to_broadcast([128, NT, CH, H]),
                                  op=mybir.AluOpType.mult)
                      if "pe" not in skip:
                          if l > 0 and "dve" not in skip:
                              # segsum absorbs the j-sum: accumulate over
                              # (tile, j) pairs straight from PMt
                              for t in range(NT):
                                  for j in range(Lc):
                                      nc.tensor.matmul(
                                          upsw[:], lhsT=s_all[:, t0 + t, :],
                                          rhs=PMt[:, t, j, :],
                                          start=(t == 0 and j == 0),
                                          stop=(t == NT - 1 and j == Lc - 1))
                          else:
                              for t in range(NT):
                                  nc.tensor.matmul(
                                      upsw[:], lhsT=s_all[:, t0 + t, :],
                                      rhs=msf[:, t, :],
                                      start=(t == 0), stop=(t == NT - 1))
                      t0 += NT

                      # ---- dense epilogue for this window ----
                      uw = pool.tile([128, C], F32, tag="uw")
                      nc.scalar.activation(out=uw[:], in_=upsw[:],
                          func=mybir.ActivationFunctionType.Copy)
                      put = psp.tile([128, C], F32, tag="pnt")
                      nc.tensor.transpose(put[:], uw[:], ident[:])
                      uc = pool.tile([128, C], F32, tag="uc")
                      nc.scalar.activation(out=uc[:], in_=put[:],
                          func=mybir.ActivationFunctionType.Copy)
                      st_w = pool.tile([1, 128], F32, tag="st_w")
                      nc.sync.dma_start(
                          out=st_w[:],
                          in_=st_d[w * 128:(w + 1) * 128][None, :])
                      pg = psp.tile([128, C], F32, tag="pm")
                      nc.tensor.matmul(pg[:], lhsT=wv_sb[:], rhs=uc[:],
                                       start=True, stop=False)
                      nc.tensor.matmul(pg[:], lhsT=bv_row[:],
                                       rhs=st_w[:],
                                       start=False, stop=True)
                      ac = pool.tile([128, C], F32, tag="ac")
                      nc.scalar.activation(out=ac[:], in_=pg[:],
                          func=mybir.ActivationFunctionType.Copy)
                      pnt2 = psp.tile([128, C], F32, tag="pnt")
                      nc.tensor.transpose(pnt2[:], ac[:], ident[:])
                      xnn = pool.tile([128, C], F32, tag="xnn")
                      nc.scalar.activation(
                          out=xnn[:], in_=pnt2[:],
                          func=mybir.ActivationFunctionType.Relu,
                          scale=dis_np[:, w:w + 1])
                      if l < L - 1:
                          xne = pool.tile([128, C], EDT, tag="xn")
                          nc.scalar.activation(out=xne[:], in_=xnn[:],
                          func=mybir.ActivationFunctionType.Copy)
                          nc.sync.dma_start(
                              out=xsl_d[w * 128:(w + 1) * 128, :], in_=xne[:])
                      # back to c-part for next layer's qt / final lin2
                      pb = psp.tile([128, C], F32, tag="pnt")
                      nc.tensor.transpose(pb[:], xnn[:], ident[:])
                      nc.scalar.activation(out=xc_all[:, w * 128:(w + 1) * 128], in_=pb[:],
                          func=mybir.ActivationFunctionType.Copy)
                  if l < L - 1:
                      if "cc" not in skip:
                          nc.gpsimd.collective_compute(
                              "AllGather", mybir.AluOpType.bypass,
                              replica_groups=groups,
                              ins=[xsl_d[:]], outs=[xf_b[:]])
                      nc.sync.dma_start(out=tq_f[:, (l + 1) * C:(l + 2) * C],
                                        in_=xf_b[:])

              # ---------- output: y = x5 @ l2w + l2b ----------
              for ch in range(NW):
                  py = psp.tile([128, DOUT], F32, tag="pm")
                  nc.tensor.matmul(py[:], lhsT=xc_all[:, ch * 128:(ch + 1) * 128],
                                   rhs=l2w_sb[:], start=True, stop=False)
                  nc.tensor.matmul(py[:], lhsT=ones_row[:], rhs=l2b_sb[:],
                                   start=False, stop=True)
                  ysb = pool.tile([128, DOUT], F32, tag="ysb")
                  nc.scalar.activation(out=ysb[:], in_=py[:],
                          func=mybir.ActivationFunctionType.Copy)
                  nc.sync.dma_start(out=y_d[ch * 128:(ch + 1) * 128, :],
                                    in_=ysb[:])

    nc.compile()
    return nc


def _prep_host(x, edge_index):
    """Shard + sort edges, build per-core index inputs."""
    row = np.concatenate([np.asarray(edge_index[0]), np.arange(N)]).astype(np.int64)
    col = np.concatenate([np.asarray(edge_index[1]), np.arange(N)]).astype(np.int64)

    # gcn_norm quantities (pure functions of edge_index)
    deg = np.zeros(N, dtype=np.float64)
    np.add.at(deg, col, 1.0)
    dis_glob = np.zeros(NP, dtype=np.float64)
    dis_glob[:N] = 1.0 / np.sqrt(deg)

    core = col // NSL
    counts = np.zeros((NCORES, NW), dtype=np.int64)
    per_core = []
    for c in range(NCORES):
        m = core == c
        rc, cc = row[m], col[m]
        o = np.argsort(cc, kind="stable")
        rc, cc = rc[o], cc[o]
        per_core.append((rc, cc))
        lw = (cc - c * NSL) // 128
        for w in range(NW):
            counts[c, w] = int((lw == w).sum())
    tiles_w = [int(np.ceil(counts[:, w].max() / 128)) for w in range(NW)]
    TOT = sum(tiles_w) * 128

    rows_p = np.zeros((NCORES, TOT), dtype=np.int64)
    cols_p = np.zeros((NCORES, TOT), dtype=np.int64)
    colw_p = np.full((NCORES, TOT), 128.0, dtype=np.float32)  # dummy -> S row 0
    disrow_p = np.zeros((NCORES, TOT), dtype=np.float32)
    st_p = np.zeros((NCORES, NSL), dtype=np.float32)
    disn_p = np.zeros((NCORES, NSL), dtype=np.float32)
    for c in range(NCORES):
        rc, cc = per_core[c]
        lc = cc - c * NSL
        np.add.at(st_p[c], lc, dis_glob[rc].astype(np.float32))
        disn_p[c] = dis_glob[c * NSL:(c + 1) * NSL]
        lw = lc // 128
        pos = 0
        for w in range(NW):
            m = lw == w
            k = int(m.sum())
            # sort within window by source row: ascending-address gathers
            o = np.argsort(rc[m], kind="stable")
            rows_p[c, pos:pos + k] = rc[m][o]
            cols_p[c, pos:pos + k] = lc[m][o]
            colw_p[c, pos:pos + k] = (lc[m][o] - w * 128).astype(np.float32)
            disrow_p[c, pos:pos + k] = dis_glob[rc[m][o]]
            pos += tiles_w[w] * 128
    return tiles_w, rows_p, cols_p, colw_p, disrow_p, st_p, disn_p


LAST_RESULTS = None


def prepare(inputs):
    """Build (nc, in_maps) for the given inputs."""
    return _prepare_impl(inputs)


def _prepare_impl(inputs):
    x = np.asarray(inputs["x"], dtype=np.float32)
    edge_index = np.asarray(inputs["edge_index"])
    lin1_w = np.asarray(inputs["lin1_w"], dtype=np.float32)
    lin1_b = np.asarray(inputs["lin1_b"], dtype=np.float32)
    Wq = np.asarray(inputs["Wq"], dtype=np.float32)
    bq = np.asarray(inputs["bq"], dtype=np.float32)
    Wk = np.asarray(inputs["Wk"], dtype=np.float32)
    Wv = np.asarray(inputs["Wv"], dtype=np.float32)
    bv = np.asarray(inputs["bv"], dtype=np.float32)
    lin2_w = np.asarray(inputs["lin2_w"], dtype=np.float32)
    lin2_b = np.asarray(inputs["lin2_b"], dtype=np.float32)

    tiles_w, rows_p, cols_p, colw_p, disrow_p, st_p, disn_p = _prep_host(
        x, edge_index)
    nc = build_program(tiles_w)

    # block-diagonal grouped weights [C, C]; wkt holds transposed blocks
    def blockdiag(W):  # W [G, CG, CG] -> [C, C]
        out = np.zeros((C, C), dtype=np.float32)
        for g in range(G):
            out[g * CG:(g + 1) * CG, g * CG:(g + 1) * CG] = W[g]
        return out

    # channel permutation: old c=(h,c2) -> new index c2*H + h, so the head
    # index is innermost on-device (packed broadcasts on DVE).  All [.,C]
    # tensors live in permuted space; weights are conjugated here.
    POS = np.array([(c % CH) * H + (c // CH) for c in range(C)])
    PM = np.zeros((C, C), dtype=np.float32)
    PM[np.arange(C), POS] = 1.0

    wq_bd = np.stack([PM.T @ blockdiag(Wq[l]) @ PM for l in range(L)])
    wkt_bd = np.stack([PM.T @ blockdiag(Wk[l].transpose(0, 2, 1)) @ PM
                       for l in range(L)])
    wv_bd = np.stack([PM.T @ blockdiag(Wv[l]) @ PM for l in range(L)])
    bq = bq @ PM
    bv = bv @ PM
    lin1_b = lin1_b @ PM
    lin2_w = PM.T @ lin2_w

    x_pad = np.zeros((NP, NFP), dtype=np.float32)
    x_pad[:N, :NF] = x
    w1_pad = np.zeros((NFP, C), dtype=np.float32)
    w1_pad[:NF] = lin1_w @ PM

    in_maps = []
    for c in range(NCORES):
        in_maps.append({
            "xsl": x_pad[c * NSL:(c + 1) * NSL],
            "rowi": _wrap_idx(rows_p[c]),
            "coli": _wrap_idx(cols_p[c]),
            "colw": colw_p[c].astype(ml_dtypes.bfloat16),
            "disrow": disrow_p[c],
            "st": st_p[c],
            "disn": disn_p[c],
            "w1": w1_pad,
            "b1": lin1_b,
            "wq": wq_bd,
            "wkt": wkt_bd,
            "wv": wv_bd,
            "bq": bq,
            "bv": bv,
            "l2w": lin2_w,
            "l2b": lin2_b,
        })

    return nc, in_maps


def assemble(res) -> np.ndarray:
    y = np.concatenate([res.results[c]["y"] for c in range(NCORES)], axis=0)
    return np.ascontiguousarray(y[:N]).astype(np.float32)


def kernel(**inputs) -> np.ndarray:
    nc, in_maps = _prepare_impl(inputs)
    res = run_bass_kernel_spmd(nc, in_maps, list(range(NCORES)))
    global LAST_RESULTS
    LAST_RESULTS = res
    return assemble(res)


if __name__ == "__main__":
    import reference
    inp = {k: np.asarray(v) for k, v in reference.setup_inputs().items()}
    out = kernel(**inp)
    print(out.shape, out.dtype)

